# revision 14
# baseline (speedup 1.0000x reference)
"""Trainium2 Bass kernel for nn_AttentionHead_17042430231165.

out = softmax(min((x@wq.T+qb)@(x@wk.T+kb).T / 256, tri)) @ (x@wv.T+vb)
  x [32, 8192], wk/wq [256, 8192], wv [8192, 8192], tri [32, 32]

Sharding (8 cores):
  - wv rows (VAL) sharded: 1024 rows/core -> each core computes out[:, c*1024:(c+1)*1024]
  - wk/wq rows (KEY) sharded: 32 dims/core -> partial scores s_c = q_c @ k_c.T [32,32]
    AllReduce(sum) of the 4KB partial-score tile gives full scores on every core.
  - x replicated. Softmax computed redundantly per core (no normalization matmul:
    out = diag(1/Z) @ (e @ v), Z = row-sums of e).

All matmuls contract over IN=8192 -> operands pre-swizzled on host to
[128 partitions, 64 chunks, cols] so every DMA is contiguous per partition.
x/wk/wq/wv are stored and streamed as bf16 (halves HBM traffic; kernel is
memory-bound); the attention epilogue (e @ v) runs as float32r (1 cycle/row).
The wv stream (16MB/core) is 8 DMAs of 4MB, triple-buffered, overlapping the
PE accumulation; the score AllReduce overlaps the stream. k and q are fused
into one 64-matmul chain (halves pre-collective PE work); only the truly
AR-dependent ops (min/exp/Z/e@v) are traced after the v-loop so the in-order
PE/DVE queues never stall stream-side work behind the collective.

Measured on trn2 (8 cores): ~112us HW exec (max core; min ~97us),
rel err ~2.4e-3 vs f32 reference.
"""
import sys

for _p in (
    "/root/.axon_site",
    "/root/.axon_site/_ro/trn_rl_repo",
    "/root/.axon_site/_ro/pypackages",
):
    if _p not in sys.path:
        sys.path.insert(0, _p)

import numpy as np
from ml_dtypes import bfloat16

from concourse import bacc, tile
from concourse import mybir
from concourse.bass_utils import run_bass_kernel_spmd

W = 32          # window (seq) size
IN = 8192       # in_size
KEY = 256       # key_size
VAL = 8192      # value_size
P = 128         # SBUF partitions
NCH = IN // P   # 64 contraction chunks
NCORES = 8
KSH = KEY // NCORES   # 32 key dims per core
VSH = VAL // NCORES   # 1024 value dims per core
SCALE = 1.0 / 256.0
CC = 8          # contraction chunks per streamed wv DMA tile (4MB bf16 per DMA)
NT = 512        # moving free dim per matmul (fp32 max)

F32 = mybir.dt.float32
F32R = mybir.dt.float32r
BF16 = mybir.dt.bfloat16

_NC = None


def _build():
    global _NC
    if _NC is not None:
        return _NC
    nc = bacc.Bacc(None, target_bir_lowering=False, debug=False, num_devices=NCORES)

    X = nc.declare_dram_parameter("X", [P, NCH, W], BF16, isOutput=False)
    WKQ = nc.declare_dram_parameter("WKQ", [P, NCH, 2 * KSH], BF16, isOutput=False)
    WV = nc.declare_dram_parameter("WV", [P, NCH, VSH], BF16, isOutput=False)
    KQB = nc.declare_dram_parameter("KQB", [2 * KSH, 1], F32, isOutput=False)
    VB = nc.declare_dram_parameter("VB", [W, VSH], F32, isOutput=False)
    TRIT = nc.declare_dram_parameter("TRIT", [W, W], F32, isOutput=False)
    OUT = nc.declare_dram_parameter("out", [W, VSH], F32, isOutput=True)

    cc_in = nc.dram_tensor("cc_in", [W, W], F32)
    cc_out = nc.dram_tensor("cc_out", [W, W], F32, addr_space="Shared")

    with tile.TileContext(nc) as tc:
        with (
            tc.tile_pool(name="const", bufs=1) as cpool,
            tc.tile_pool(name="wv", bufs=3) as wpool,
            tc.tile_pool(name="small", bufs=1) as spool,
            tc.tile_pool(name="psum", bufs=1, space="PSUM") as ppool,
        ):
            # constants / replicated inputs
            x_sb = cpool.tile([P, NCH, W], BF16)
            nc.sync.dma_start(out=x_sb[:], in_=X[:])
            wkq_sb = cpool.tile([P, NCH, 2 * KSH], BF16)
            nc.sync.dma_start(out=wkq_sb[:], in_=WKQ[:])
            kqb_sb = cpool.tile([2 * KSH, 1], F32)
            nc.gpsimd.dma_start(out=kqb_sb[:], in_=KQB[:])
            vb_sb = cpool.tile([W, VSH], F32)
            nc.gpsimd.dma_start(out=vb_sb[:], in_=VB[:])
            trit_sb = cpool.tile([W, W], F32)
            nc.gpsimd.dma_start(out=trit_sb[:], in_=TRIT[:])
            ones_w = cpool.tile([W, 1], F32)
            nc.vector.memset(ones_w[:], 1.0)

            # kqT = [wk; wq] @ x.T + [kb; qb]  [2*KSH, W] in one chain
            pkq = ppool.tile([2 * KSH, W], F32)
            for c in range(NCH):
                nc.tensor.matmul(
                    pkq[:], wkq_sb[:, c, :], x_sb[:, c, :],
                    start=(c == 0), stop=(c == NCH - 1),
                )
            kqT = spool.tile([2 * KSH, W], F32)
            nc.vector.tensor_scalar_add(kqT[:], pkq[:], kqb_sb[:])
            # q half copied to a base-0 tile (matmul needs equal base partitions)
            qT = spool.tile([KSH, W], F32)
            nc.gpsimd.dma_start(out=qT[:], in_=kqT[KSH:2 * KSH, :])

            # partial scores sT[j, i] = sum_d k[j,d] q[i,d]  (this core's d-slice)
            ps = ppool.tile([W, W], F32)
            nc.tensor.matmul(ps[:], kqT[0:KSH, :], qT[:])
            s_sb = spool.tile([W, W], F32)
            nc.scalar.mul(s_sb[:], ps[:], SCALE)

            # AllReduce the partial scores across 8 cores (4KB)
            nc.gpsimd.dma_start(out=cc_in[:], in_=s_sb[:])
            nc.gpsimd.collective_compute(
                "AllReduce",
                mybir.AluOpType.add,
                replica_groups=[list(range(NCORES))],
                ins=[cc_in.ap().opt()],
                outs=[cc_out.ap().opt()],
            )
            S_sb = spool.tile([W, W], F32)
            nc.gpsimd.dma_start(out=S_sb[:], in_=cc_out[:])

            # v = x @ wv_c.T streamed over 64 contraction chunks
            pv0 = ppool.tile([W, NT], F32)
            pv1 = ppool.tile([W, NT], F32)
            for d in range(NCH // 8):
                wt = wpool.tile([P, 8, VSH], BF16, tag="wvstream")
                nc.sync.dma_start(out=wt[:], in_=WV[:, d * 8:(d + 1) * 8, :])
                for i in range(8):
                    c = d * 8 + i
                    nc.tensor.matmul(
                        pv0[:], x_sb[:, c, :], wt[:, i, 0:NT],
                        start=(c == 0), stop=(c == NCH - 1),
                    )
                    nc.tensor.matmul(
                        pv1[:], x_sb[:, c, :], wt[:, i, NT:VSH],
                        start=(c == 0), stop=(c == NCH - 1),
                    )
            # v copies depend only on the stream -> trace them BEFORE the
            # AR-dependent ops so the in-order DVE/PE queues don't stall the
            # stream-side work behind the collective.
            v_sbs = []
            for j, pv in enumerate((pv0, pv1)):
                v_sb = spool.tile([W, NT], F32R, tag=f"v{j}")
                nc.vector.tensor_copy(v_sb[:], pv[:])
                v_sbs.append(v_sb)

            # AR-dependent epilogue: eT = exp(min(S, triT)); Z via ones-matmul
            m_sb = spool.tile([W, W], F32)
            nc.vector.tensor_tensor(m_sb[:], S_sb[:], trit_sb[:], mybir.AluOpType.min)
            e_sb = spool.tile([W, W], F32R)
            nc.scalar.activation(e_sb[:], m_sb[:], mybir.ActivationFunctionType.Exp)
            e32 = spool.tile([W, W], F32)
            nc.scalar.activation(e32[:], m_sb[:], mybir.ActivationFunctionType.Exp)
            pz = ppool.tile([W, 1], F32)
            nc.tensor.matmul(pz[:], e32[:], ones_w[:])
            rz = spool.tile([W, 1], F32)
            nc.vector.reciprocal(rz[:], pz[:])

            for j, v_sb in enumerate(v_sbs):
                pu = ppool.tile([W, NT], F32, tag=f"pu{j}")
                nc.tensor.matmul(pu[:], e_sb[:], v_sb[:])
                o_sb = spool.tile([W, NT], F32, tag=f"o{j}")
                nc.vector.scalar_tensor_tensor(
                    o_sb[:], pu[:], rz[:], vb_sb[:, j * NT:(j + 1) * NT],
                    mybir.AluOpType.mult, mybir.AluOpType.add,
                )
                nc.gpsimd.dma_start(out=OUT[:, j * NT:(j + 1) * NT], in_=o_sb[:])

    nc.compile()
    _NC = nc
    return nc


def _swizzle(mat_t):
    """[rows=IN, cols] (transposed so IN is dim 0) -> bf16 [P, NCH, cols]."""
    rows, cols = mat_t.shape
    assert rows == IN
    return np.ascontiguousarray(
        mat_t.reshape(NCH, P, cols).transpose(1, 0, 2).astype(bfloat16))


def _make_in_maps(x, wk_w, wk_b, wq_w, wq_b, wv_w, wv_b, tri):
    x = np.asarray(x, dtype=np.float32)
    X_dev = _swizzle(np.ascontiguousarray(x.T))
    TRIT = np.ascontiguousarray(np.asarray(tri, dtype=np.float32).T)
    in_maps = []
    for c in range(NCORES):
        wk_sh = np.asarray(wk_w[c * KSH:(c + 1) * KSH, :], dtype=np.float32)
        wq_sh = np.asarray(wq_w[c * KSH:(c + 1) * KSH, :], dtype=np.float32)
        wv_sh = np.asarray(wv_w[c * VSH:(c + 1) * VSH, :], dtype=np.float32)
        wv_sw = _swizzle(np.ascontiguousarray(wv_sh.T))
        in_maps.append({
            "X": X_dev,
            "WKQ": _swizzle(np.ascontiguousarray(
                np.concatenate([wk_sh, wq_sh], axis=0).T)),
            "WV": wv_sw,
            "KQB": np.ascontiguousarray(np.concatenate([
                np.asarray(wk_b[c * KSH:(c + 1) * KSH], dtype=np.float32),
                np.asarray(wq_b[c * KSH:(c + 1) * KSH], dtype=np.float32),
            ]).reshape(2 * KSH, 1)),
            "VB": np.ascontiguousarray(np.broadcast_to(
                np.asarray(wv_b[c * VSH:(c + 1) * VSH], dtype=np.float32).reshape(1, VSH),
                (W, VSH))),
            "TRIT": TRIT,
        })
    return in_maps


def run(inputs, trace=False):
    """Build + run on 8 cores; returns (full_output, BassKernelResults)."""
    nc = _build()
    in_maps = _make_in_maps(**inputs)
    res = run_bass_kernel_spmd(
        nc, in_maps, core_ids=list(range(NCORES)), trace=trace,
    )
    out = np.concatenate([res.results[c]["out"] for c in range(NCORES)], axis=1)
    return out, res


def kernel(**inputs):
    out, _ = run(inputs, trace=False)
    return out


if __name__ == "__main__":
    rng = np.random.default_rng(0)
    ins = {
        "x": rng.standard_normal((W, IN), dtype=np.float32),
        "wk_w": rng.standard_normal((KEY, IN), dtype=np.float32) / 90.5,
        "wk_b": rng.standard_normal((KEY,), dtype=np.float32) / 90.5,
        "wq_w": rng.standard_normal((KEY, IN), dtype=np.float32) / 90.5,
        "wq_b": rng.standard_normal((KEY,), dtype=np.float32) / 90.5,
        "wv_w": rng.standard_normal((VAL, IN), dtype=np.float32) / 90.5,
        "wv_b": rng.standard_normal((VAL,), dtype=np.float32) / 90.5,
        "tri": ((np.tril(np.full((W, W), 2.0, dtype=np.float32)) - 1.0) * 1e5),
    }
    out = kernel(**ins)
    print("out", out.shape, out.dtype, np.abs(out).mean())
